# revision 21
# baseline (speedup 1.0000x reference)
"""Multi-head attention (B=4, S=2048, D=1024, 16 heads x 64) on 8 NeuronCores.

Sharding: DP=4 over batch x TP=2 over heads (8 heads/core).

v3: bf16 matmuls (baseline numerics) with a qc-major schedule: all four
head-pair blocks for one 512-query chunk run back-to-back, so each chunk's
fc (output projection) chains ride as PE fillers inside the NEXT sweep
instead of bunching at the kernel tail. Milestone-based filler draining
guarantees producer chains (k/q/v projections) are emitted before their
consumers. Per core (1 batch element, 8 heads):

    qhT/khT = (x @ W) + b      bf16 [128, 2048] per head-pair
    vh      = v @ Wv           bf16 j-pair tiles [128, 2x(8x66)] w/ones col
    ST      = kh @ qh^T per j  -> psum [128, 1024] (2 packed K=64 MMs)
    EXP     -> bf16 p-pair tiles [128, (h j n)] (strided ACT output)
    PV      per j: lhsT=vh[128,65], rhs=p[128,512] x 2 heads; row 64
             accumulates the softmax denominator L (ones column)
    norm    = outT -> bf16 hp-pair tiles [128, 2x2048]
    fc      = 4 accum MMs over hp -> psum -> bf16 out

Inputs land in 2-slot sc rings for q and v (halves SBUF); host sums the TP
pair partials and adds the bias terms (bv @ Wo + bo).
"""

import os
import sys

sys.path.insert(0, "/opt/trn_rl_repo")

import numpy as np
import ml_dtypes

S = 2048          # sequence length
DM = 1024         # model dim
HD = 512          # local head-dim total (8 heads x 64) per core (TP=2)
NB = 4            # batch
NCORES = 8
P = 128
DK = 64
HW8 = 128         # per-head vh slot: 64 dims + ones col + zero pad to 128
VW = 8 * HW8      # 1024 per j-chunk half (128-col slots keep FWL eligible)
SCALE = 1.0 / 8.0  # 1/sqrt(64)

NM = DM // P       # 8 contraction chunks of 128
NMP = NM // 2      # 4 chunk-pairs (tile granularity)
NHP = HD // P      # 4 head pairs
NSC = S // 512     # 4 s-chunks of 512
NJ = S // P        # 16 key chunks
NJP = NJ // 2      # 8 key chunk-pairs

BF16 = ml_dtypes.bfloat16

_CACHE = {}


def _build_nc():
    import concourse.bass as bass  # noqa: F401
    import concourse.mybir as mybir
    from concourse import bacc, tile
    from contextlib import ExitStack

    BF = mybir.dt.bfloat16
    F32 = mybir.dt.float32
    Exp = mybir.ActivationFunctionType.Exp

    nc = bacc.Bacc("TRN2", target_bir_lowering=False, debug=False, num_swdge_queues=4)

    # pair-layout inputs: row mp*128+p, col sc*1024 + i*512 + t
    qp = nc.dram_tensor("qp", [HD, 4096], BF, kind="ExternalInput")
    kp = nc.dram_tensor("kp", [HD, 4096], BF, kind="ExternalInput")
    vp = nc.dram_tensor("vp", [HD, 4096], BF, kind="ExternalInput")
    # weights, pair layout [mp*128+p, i*512 + h]
    wq = nc.dram_tensor("wq", [HD, 1024], BF, kind="ExternalInput")
    wk = nc.dram_tensor("wk", [HD, 1024], BF, kind="ExternalInput")
    wv = nc.dram_tensor("wv", [HD, 1024], BF, kind="ExternalInput")
    # wo pair layout [hpp*128+p, i*1024 + d]
    wo = nc.dram_tensor("wo", [256, 2048], BF, kind="ExternalInput")
    bq = nc.dram_tensor("bq", [HD], F32, kind="ExternalInput")
    bk = nc.dram_tensor("bk", [HD], F32, kind="ExternalInput")
    out = nc.dram_tensor("out", [S, DM], BF, kind="ExternalOutput")

    with ExitStack() as ctx:
        tc = ctx.enter_context(tile.TileContext(nc))

        const = ctx.enter_context(tc.tile_pool(name="const", bufs=1))
        w_pool = ctx.enter_context(tc.tile_pool(name="w_pool", bufs=4))
        wo_pool = ctx.enter_context(tc.tile_pool(name="wo_pool", bufs=2))
        kin_pool = ctx.enter_context(tc.tile_pool(name="kin_pool", bufs=16))
        qin_pool = ctx.enter_context(tc.tile_pool(name="qin_pool", bufs=8))
        vin_pool = ctx.enter_context(tc.tile_pool(name="vin_pool", bufs=8))
        qh_pool = ctx.enter_context(tc.tile_pool(name="qh_pool", bufs=4))
        vh_pool = ctx.enter_context(tc.tile_pool(name="vh_pool", bufs=8))
        outT_pool = ctx.enter_context(tc.tile_pool(name="outT_pool", bufs=2))
        p_pool = ctx.enter_context(tc.tile_pool(name="p_pool", bufs=4))
        rec_pool = ctx.enter_context(tc.tile_pool(name="rec_pool", bufs=1))
        recb_pool = ctx.enter_context(tc.tile_pool(name="recb_pool", bufs=1))
        tmp_pool = ctx.enter_context(tc.tile_pool(name="tmp_pool", bufs=1))
        stage_pool = ctx.enter_context(tc.tile_pool(name="stage_pool", bufs=2))
        st_ps = ctx.enter_context(tc.tile_pool(name="st_ps", bufs=2, space="PSUM"))
        chain_ps = ctx.enter_context(tc.tile_pool(name="chain_ps", bufs=2, space="PSUM"))
        pv_ps = ctx.enter_context(tc.tile_pool(name="pv_ps", bufs=2, space="PSUM"))

        # constants: one-hot row selector (row 64 -> broadcast rec to 64 rows)
        sel_row = const.tile([P, P], BF, tag="sel", name="sel_row")
        nc.vector.memset(sel_row[:], 0.0)
        nc.vector.memset(sel_row[DK : DK + 1, 0:DK], 1.0)
        recbf_tile = rec_pool.tile([P, 1024], BF, tag="recbf", name="recbf")
        nc.vector.memset(recbf_tile[:], 0.0)
        bq_sb = const.tile([P, NHP], F32, tag="bq", name="bq_sb")
        nc.gpsimd.dma_start(bq_sb[:], bq[:].rearrange("(f p) -> p f", p=P))
        bk_sb = const.tile([P, NHP], F32, tag="bk", name="bk_sb")
        nc.gpsimd.dma_start(bk_sb[:], bk[:].rearrange("(f p) -> p f", p=P))

        # ---- SBUF input tiles [128, 2x512] bf16; k full, q/v 2-slot sc rings
        kin = [
            [
                kin_pool.tile([P, 1024], BF, tag="kin", name=f"k_{mp}_{sc}")
                for sc in range(NSC)
            ]
            for mp in range(NMP)
        ]
        qin = [
            [
                qin_pool.tile([P, 1024], BF, tag="qin", name=f"q_{mp}_{sl}")
                for sl in range(2)
            ]
            for mp in range(NMP)
        ]
        vin = [
            [
                vin_pool.tile([P, 1024], BF, tag="vin", name=f"v_{mp}_{sl}")
                for sl in range(2)
            ]
            for mp in range(NMP)
        ]

        def emit_in_dmas(tiles, handle, sc, eng, ring=False):
            sl = sc % 2 if ring else sc
            for mp in range(NMP):
                eng.dma_start(
                    tiles[mp][sl][:],
                    handle[mp * P : (mp + 1) * P, sc * 1024 : (sc + 1) * 1024],
                )

        def load_w(handle, tag, eng):
            tiles = []
            for mp in range(NMP):
                t = w_pool.tile([P, 1024], BF, tag=tag, name=f"{tag}{mp}")
                eng.dma_start(t[:], handle[mp * P : (mp + 1) * P, :])
                tiles.append(t)
            return tiles

        _q = [nc.gpsimd, nc.scalar, nc.sync]
        _qi = [0]

        def nxq():
            _qi[0] = (_qi[0] + 1) % 3
            return _q[_qi[0]]

        # demand-ordered input DMA (first block needs wk+kp0, wq+qp0, wv+vp0);
        # q/v rings: slots for sc 0/1 upfront, sc 2/3 ride in fillers
        wk_sb = load_w(wk, "wk", nxq())
        emit_in_dmas(kin, kp, 0, nxq())
        wq_sb = load_w(wq, "wq", nxq())
        emit_in_dmas(qin, qp, 0, nxq(), ring=True)
        wv_sb = load_w(wv, "wv", nxq())
        emit_in_dmas(vin, vp, 0, nxq(), ring=True)
        emit_in_dmas(kin, kp, 1, nxq())
        emit_in_dmas(vin, vp, 1, nxq(), ring=True)
        emit_in_dmas(kin, kp, 2, nxq())
        emit_in_dmas(kin, kp, 3, nxq())
        emit_in_dmas(qin, qp, 1, nxq(), ring=True)
        wo_sb = []
        for hpp in range(2):
            t = wo_pool.tile([P, 2048], BF, tag="wo", name=f"wo{hpp}")
            nxq().dma_start(t[:], wo[hpp * P : (hpp + 1) * P, :])
            wo_sb.append(t)

        def pair2(t):
            return t[:].rearrange("p (two c) -> p two c", two=2)

        # ---- vh j-pair tiles [128, 2 x (8 x 66)] bf16, ones at slot col 64
        vh_sb = [
            vh_pool.tile([P, 2 * VW], BF, tag="vh", name=f"vh{i}") for i in range(NJP)
        ]

        def vh4(i):
            return vh_sb[i][:].rearrange("p (two h m) -> p two h m", two=2, h=8)

        for i in range(NJP):
            nc.vector.memset(vh_sb[i][:], 0.0)
            nc.vector.memset(vh4(i)[:, :, :, DK : DK + 1], 1.0)

        def vproj_chain_ops(j):
            sc, half, tq = j // 4, j % 2, (j % 4) * P
            vsl = sc % 2
            cell = {}

            def mk(mp, i):
                def op():
                    if mp == 0 and i == 0:
                        cell["ps"] = chain_ps.tile([P, HD], F32, tag="chps", name="vps")
                    nc.tensor.matmul(
                        cell["ps"][:],
                        lhsT=pair2(vin[mp][vsl])[:, i, tq : tq + P],
                        rhs=pair2(wv_sb[mp])[:, i, :],
                        start=(mp == 0 and i == 0),
                        stop=(mp == NMP - 1 and i == 1),
                    )
                return op

            ops = [mk(mp, i) for mp in range(NMP) for i in range(2)]

            def ev():
                nc.vector.tensor_copy(
                    vh4(j // 2)[:, half, :, 0:DK],
                    cell["ps"][:].rearrange("p (h m) -> p h m", h=8),
                )

            ops.append(ev)
            return ops

        # ---- q/k projections: full bf16 tiles for all head-pairs ----
        qh_sb = [qh_pool.tile([P, S], BF, tag="qh", name=f"qh{h}") for h in range(NHP)]
        kh_sb = [qh_pool.tile([P, S], BF, tag="kh", name=f"kh{h}") for h in range(NHP)]

        def proj_chain_ops(w_sb, x_sb, dst, bias_sb, hp, sc, ring):
            xsl = sc % 2 if ring else sc
            cell = {}

            def mk(mp, i):
                def op():
                    if mp == 0 and i == 0:
                        cell["ps"] = chain_ps.tile([P, HD], F32, tag="chps", name="fps")
                    nc.tensor.matmul(
                        cell["ps"][:],
                        lhsT=pair2(w_sb[mp])[:, i, hp * P : (hp + 1) * P],
                        rhs=pair2(x_sb[mp][xsl])[:, i, :],
                        start=(mp == 0 and i == 0),
                        stop=(mp == NMP - 1 and i == 1),
                    )
                return op

            ops = [mk(mp, i) for mp in range(NMP) for i in range(2)]

            def ev():
                nc.vector.tensor_scalar_add(
                    dst[:, sc * 512 : (sc + 1) * 512],
                    cell["ps"][:],
                    bias_sb[:, hp : hp + 1],
                )

            ops.append(ev)
            return ops

        # ---- outT hp-pair tiles [128, 2 x 2048] bf16 ----
        outT_sb = [
            outT_pool.tile([P, 2 * S], BF, tag="outT", name=f"outT{i}")
            for i in range(2)
        ]

        def fc_chain_ops(ss, ec):
            tsl = slice(ss * P, (ss + 1) * P)
            es = slice(ec * 512, (ec + 1) * 512)
            cell = {}

            def mk(hpp, i):
                def op():
                    if hpp == 0 and i == 0:
                        cell["ps"] = chain_ps.tile([P, HD], F32, tag="chps", name="fcps")
                    nc.tensor.matmul(
                        cell["ps"][:],
                        lhsT=pair2(outT_sb[hpp])[:, i, tsl],
                        rhs=pair2(wo_sb[hpp])[:, i, es],
                        start=(hpp == 0 and i == 0),
                        stop=(hpp == 1 and i == 1),
                    )
                return op

            ops = [mk(hpp, i) for hpp in range(2) for i in range(2)]

            def ev():
                stg = stage_pool.tile([P, 512], BF, tag="stg", name="stg")
                nc.vector.tensor_copy(stg[:], cell["ps"][:])
                nc.gpsimd.dma_start(out[tsl, es], stg[:])

            ops.append(ev)
            return ops

        from collections import deque

        fillers = deque()  # items: (key, op, is_last)
        done = set()

        def push(key, ops):
            for i, op in enumerate(ops):
                fillers.append((key, op, i == len(ops) - 1))

        def drain(n):
            for _ in range(n):
                if not fillers:
                    return
                key, op, last = fillers.popleft()
                op()
                if last:
                    done.add(key)

        def require(key):
            while key not in done and fillers:
                drain(1)

        def kh_chain(hp, sc):
            return proj_chain_ops(wk_sb, kin, kh_sb[hp], bk_sb, hp, sc, False)

        def qh_chain(hp, qc):
            return proj_chain_ops(wq_sb, qin, qh_sb[hp], bq_sb, hp, qc, True)

        # upfront: what block (0,0) needs to start
        for key, ops in (
            (("kh", 0, 0), kh_chain(0, 0)),
            (("qh", 0, 0), qh_chain(0, 0)),
        ):
            for op in ops:
                op()
            done.add(key)

        # fillers: kh for all hps + vproj in demand order, then qh hp1-3 (qc0).
        # v ring: the slot for sc 2/3 is DMA'd after the chains reading it.
        push(("vp", 0), vproj_chain_ops(0))
        push(("vp", 1), vproj_chain_ops(1))
        push(("kh", 0, 1), kh_chain(0, 1))
        push(("vp", 2), vproj_chain_ops(2))
        push(("kh", 0, 2), kh_chain(0, 2))
        push(("vp", 3), vproj_chain_ops(3))
        push(("kh", 0, 3), kh_chain(0, 3))
        push(("vp", 4), vproj_chain_ops(4))
        push(("kh", 1, 0), kh_chain(1, 0))
        push(("vp", 5), vproj_chain_ops(5))
        push(("kh", 1, 1), kh_chain(1, 1))
        push(("vp", 6), vproj_chain_ops(6))
        push(("vp", 7), vproj_chain_ops(7))
        push(("vdma", 2), [lambda: emit_in_dmas(vin, vp, 2, nc.gpsimd, ring=True)])
        push(("kh", 1, 2), kh_chain(1, 2))
        push(("vp", 8), vproj_chain_ops(8))
        push(("kh", 1, 3), kh_chain(1, 3))
        push(("vp", 9), vproj_chain_ops(9))
        push(("kh", 2, 0), kh_chain(2, 0))
        push(("vp", 10), vproj_chain_ops(10))
        push(("vp", 11), vproj_chain_ops(11))
        push(("vdma", 3), [lambda: emit_in_dmas(vin, vp, 3, nc.gpsimd, ring=True)])
        push(("kh", 2, 1), kh_chain(2, 1))
        push(("vp", 12), vproj_chain_ops(12))
        push(("kh", 2, 2), kh_chain(2, 2))
        push(("vp", 13), vproj_chain_ops(13))
        push(("kh", 2, 3), kh_chain(2, 3))
        push(("vp", 14), vproj_chain_ops(14))
        push(("kh", 3, 0), kh_chain(3, 0))
        push(("vp", 15), vproj_chain_ops(15))
        push(("kh", 3, 1), kh_chain(3, 1))
        push(("kh", 3, 2), kh_chain(3, 2))
        push(("kh", 3, 3), kh_chain(3, 3))
        push(("qh", 1, 0), qh_chain(1, 0))
        push(("qh", 2, 0), qh_chain(2, 0))
        push(("qh", 3, 0), qh_chain(3, 0))
        # q ring: the sc=2 slot frees once the qh(*, 0) chains are done
        push(("qdma", 2), [lambda: emit_in_dmas(qin, qp, 2, nc.gpsimd, ring=True)])

        # ---- attention block: ST bf16 pairs + EXP + PV, fillers between ----
        carry = []

        def attn_block(hp, qc, budget, lagp=2):
            for sc in range(NSC):
                require(("kh", hp, sc))
            require(("qh", hp, qc))
            qs = slice(qc * 512, (qc + 1) * 512)
            state = {}
            p_tiles = {}

            def emit_st(j):
                ks = slice(j * P, (j + 1) * P)
                st = st_ps.tile([P, 1024], F32, tag="stps", name="stps")
                nc.tensor.matmul(
                    st[:, 0:512],
                    lhsT=kh_sb[hp][0:64, ks],
                    rhs=qh_sb[hp][0:64, qs],
                    start=True,
                    stop=True,
                    tile_position=(0, 0),
                )
                nc.tensor.matmul(
                    st[:, 512:1024],
                    lhsT=kh_sb[hp][64:128, ks],
                    rhs=qh_sb[hp][64:128, qs],
                    start=True,
                    stop=True,
                    tile_position=(64, 0),
                )
                jp, jh = j // 2, j % 2
                if jh == 0:
                    p_tiles[jp] = p_pool.tile([P, 2048], BF, tag="p", name="ppair")
                pt4 = p_tiles[jp][:].rearrange("p (h j n) -> p h j n", h=2, j=2)
                nc.scalar.activation(
                    pt4[:, :, jh, :],
                    st[:].rearrange("p (h n) -> p h n", h=2),
                    Exp,
                    scale=SCALE,
                )

            def emit_pv(jp):
                require(("vp", 2 * jp))
                require(("vp", 2 * jp + 1))
                if "P0" not in state:
                    state["P0"] = pv_ps.tile([P, 512], F32, tag="pvps", name="P0ps")
                    state["P1"] = pv_ps.tile([P, 512], F32, tag="pvps", name="P1ps")
                pt = p_tiles.pop(jp)
                pt4 = pt[:].rearrange("p (h j n) -> p h j n", h=2, j=2)
                for i in range(2):
                    first, last = (jp == 0 and i == 0), (jp == NJP - 1 and i == 1)
                    for h in range(2):
                        nc.tensor.matmul(
                            state["P0" if h == 0 else "P1"][:, :],
                            lhsT=vh4(jp)[:, i, 2 * hp + h, :],
                            rhs=pt4[:, h, i, :],
                            start=first,
                            stop=last,
                            skip_group_check=True,
                        )

            for i in range(NJP):
                # both j's ST pairs back-to-back: one contiguous K=64 tile
                # stretch per pair-step halves the PE 64<->128 mode switches
                emit_st(2 * i)
                emit_st(2 * i + 1)
                for _ in range(4):
                    if carry:
                        op = carry.pop(0)
                        if op is not None:
                            op()
                if i >= lagp:
                    emit_pv(i - lagp)
                if not carry:
                    drain(budget[i] if isinstance(budget, list) else budget)

            def mk_pv(jp):
                return lambda: emit_pv(jp)

            def mk_norm():
                cellN = {}
                half = hp % 2
                ot = outT_sb[hp // 2]

                def ot2(lo, hi):
                    return (
                        ot[:]
                        .rearrange("p (two s) -> p two s", two=2)[lo:hi, half, qs]
                    )

                def evac_p0():
                    E0 = tmp_pool.tile([P, 512], F32, tag="E0", name="E0")
                    cellN["E0"] = E0
                    nc.vector.tensor_copy(E0[0 : DK + 1, :], state["P0"][0 : DK + 1, :])

                def evac_p1():
                    E1 = tmp_pool.tile([P, 512], F32, tag="E1", name="E1")
                    cellN["E1"] = E1
                    nc.vector.tensor_copy(E1[0 : DK + 1, :], state["P1"][0 : DK + 1, :])

                def recips():
                    # full [0:65] so the custom-DVE op starts at partition 0;
                    # rows 0:64 compute junk reciprocals that are never read
                    rec = rec_pool.tile([P, 1024], F32, tag="rec", name="rec")
                    cellN["rec"] = rec
                    nc.vector.reciprocal_approx_fast(
                        rec[0 : DK + 1, 0:512], cellN["E0"][0 : DK + 1, :]
                    )
                    nc.vector.reciprocal_approx_fast(
                        rec[0 : DK + 1, 512:1024], cellN["E1"][0 : DK + 1, :]
                    )

                def to_bf():
                    nc.vector.tensor_copy(
                        recbf_tile[DK : DK + 1, :], cellN["rec"][DK : DK + 1, :]
                    )

                def bcast_mm():
                    r0 = chain_ps.tile([P, HD], F32, tag="chps", name="rb0")
                    r1 = chain_ps.tile([P, HD], F32, tag="chps", name="rb1")
                    cellN["rb0"], cellN["rb1"] = r0, r1
                    nc.tensor.matmul(
                        r0[:], lhsT=sel_row[:], rhs=recbf_tile[:, 0:512],
                        start=True, stop=True,
                    )
                    nc.tensor.matmul(
                        r1[:], lhsT=sel_row[:], rhs=recbf_tile[:, 512:1024],
                        start=True, stop=True,
                    )

                def evac_recb():
                    recb_sb = recb_pool.tile([P, 1024], BF, tag="recb", name="recb")
                    cellN["recb"] = recb_sb
                    nc.vector.tensor_copy(recb_sb[0:DK, 0:512], cellN["rb0"][0:DK, :])
                    nc.vector.tensor_copy(
                        recb_sb[0:DK, 512:1024], cellN["rb1"][0:DK, :]
                    )

                def mul_e():
                    nc.vector.tensor_mul(
                        ot2(0, DK),
                        cellN["E0"][0:DK, :],
                        cellN["recb"][0:DK, 0:512],
                    )

                def mul_o():
                    tmp = tmp_pool.tile([P, 512], BF, tag="tmp", name="tmp")
                    cellN["tmp"] = tmp
                    nc.vector.tensor_mul(
                        tmp[0:DK, :],
                        cellN["E1"][0:DK, :],
                        cellN["recb"][0:DK, 512:1024],
                    )

                def shift():
                    nc.sync.dma_start(ot2(DK, P), cellN["tmp"][0:DK, :])

                return [
                    evac_p0,
                    evac_p1,
                    recips,
                    None,
                    to_bf,
                    None,
                    None,
                    bcast_mm,
                    evac_recb,
                    mul_e,
                    mul_o,
                    shift,
                ]

            return [mk_pv(jp) for jp in range(NJP - lagp, NJP)] + mk_norm()

        for qc in range(NSC):
            for hp in range(NHP):
                if qc == 0 and hp == 0:
                    # DMA-limited ramp: hold fillers while inputs land
                    carry = attn_block(hp, qc, [0, 0, 2, 4, 7, 7, 8, 8], lagp=3)
                else:
                    carry = attn_block(hp, qc, 8 if qc == 0 else 2)
                if hp == 0 and qc > 0:
                    # previous qc's outT is complete; queue its fc chains
                    for ss in range((qc - 1) * 4, (qc - 1) * 4 + 4):
                        push(("fc", ss, 0), fc_chain_ops(ss, 0))
                        push(("fc", ss, 1), fc_chain_ops(ss, 1))
                if qc < NSC - 1:
                    push(("qh", hp, qc + 1), qh_chain(hp, qc + 1))
                    if hp == NHP - 1 and qc + 3 <= NSC - 1:
                        # q ring: the slot for sc=qc+3 frees after qh(*, qc+1)
                        push(
                            ("qdma", qc + 3),
                            [
                                lambda sc=qc + 3: emit_in_dmas(
                                    qin, qp, sc, nc.gpsimd, ring=True
                                )
                            ],
                        )
        for op in carry:
            if op is not None:
                op()
        for ss in range(12, 16):
            push(("fc", ss, 0), fc_chain_ops(ss, 0))
            push(("fc", ss, 1), fc_chain_ops(ss, 1))
        while fillers:
            drain(1)

    nc.compile()
    return nc


def _get_nc():
    if "nc" not in _CACHE:
        _CACHE["nc"] = _build_nc()
    return _CACHE["nc"]


def _pair_inputs(xT):
    # [1024, 2048] -> [512, 4096]: row mp*128+p, col sc*1024 + i*512 + t
    return (
        xT.reshape(NMP, 2, P, NSC, 512)
        .transpose(0, 2, 3, 1, 4)
        .reshape(HD, 4096)
        .astype(BF16)
    )


def kernel(q, k, v, Wq, bq, Wk, bk, Wv, bv, Wo, bo):
    from concourse.bass_utils import run_bass_kernel_spmd

    q, k, v = (np.asarray(x, np.float32) for x in (q, k, v))
    Wq, bq, Wk, bk, Wv, bv, Wo, bo = (
        np.asarray(x, np.float32) for x in (Wq, bq, Wk, bk, Wv, bv, Wo, bo)
    )

    def wpair(W, t):
        Ws = W[:, t * HD : (t + 1) * HD]
        return (
            Ws.reshape(NMP, 2, P, HD).transpose(0, 2, 1, 3).reshape(HD, 1024).astype(BF16)
        )

    def wopair(t):
        Ws = Wo[t * HD : (t + 1) * HD, :]
        return (
            Ws.reshape(2, 2, P, DM).transpose(0, 2, 1, 3).reshape(256, 2048).astype(BF16)
        )

    in_maps = []
    for c in range(NCORES):
        b, t = c // 2, c % 2
        hs = slice(t * HD, (t + 1) * HD)
        in_maps.append(
            {
                "qp": _pair_inputs(q[b].T),
                "kp": _pair_inputs(k[b].T),
                "vp": _pair_inputs(v[b].T),
                "wq": wpair(Wq, t),
                "wk": wpair(Wk, t),
                "wv": wpair(Wv, t),
                "wo": wopair(t),
                "bq": np.ascontiguousarray(bq[hs]),
                "bk": np.ascontiguousarray(bk[hs]),
            }
        )

    nc = _get_nc()
    trace = os.environ.get("KERNEL_TRACE", "0") == "1"
    res = run_bass_kernel_spmd(
        nc, in_maps, core_ids=list(range(NCORES)), trace=trace
    )
    if trace:
        print(f"HW exec time: {res.exec_time_ns} ns")

    host_bias = (bv @ Wo + bo).astype(np.float32)
    full = np.empty((NB, S, DM), np.float32)
    for b in range(NB):
        full[b] = (
            res.results[2 * b]["out"].astype(np.float32)
            + res.results[2 * b + 1]["out"].astype(np.float32)
            + host_bias
        )
    return full


# revision 22
# speedup vs baseline: 1.0224x; 1.0224x over previous
"""Multi-head attention (B=4, S=2048, D=1024, 16 heads x 64) on 8 NeuronCores.

Sharding: DP=4 over batch x TP=2 over heads (8 heads/core).

v3: bf16 matmuls (baseline numerics) with a qc-major schedule: all four
head-pair blocks for one 512-query chunk run back-to-back, so each chunk's
fc (output projection) chains ride as PE fillers inside the NEXT sweep
instead of bunching at the kernel tail. Milestone-based filler draining
guarantees producer chains (k/q/v projections) are emitted before their
consumers. Per core (1 batch element, 8 heads):

    qhT/khT = (x @ W) + b      bf16 [128, 2048] per head-pair
    vh      = v @ Wv           bf16 j-pair tiles [128, 2x(8x66)] w/ones col
    ST      = kh @ qh^T per j  -> psum [128, 1024] (2 packed K=64 MMs)
    EXP     -> bf16 p-pair tiles [128, (h j n)] (strided ACT output)
    PV      per j: lhsT=vh[128,65], rhs=p[128,512] x 2 heads; row 64
             accumulates the softmax denominator L (ones column)
    norm    = outT -> bf16 hp-pair tiles [128, 2x2048]
    fc      = 4 accum MMs over hp -> psum -> bf16 out

Inputs land in 2-slot sc rings for q and v (halves SBUF); host sums the TP
pair partials and adds the bias terms (bv @ Wo + bo).
"""

import os
import sys

sys.path.insert(0, "/opt/trn_rl_repo")

import numpy as np
import ml_dtypes

S = 2048          # sequence length
DM = 1024         # model dim
HD = 512          # local head-dim total (8 heads x 64) per core (TP=2)
NB = 4            # batch
NCORES = 8
P = 128
DK = 64
HW8 = 128         # per-head vh slot: 64 dims + ones col + zero pad to 128
VW = 8 * HW8      # 1024 per j-chunk half (128-col slots keep FWL eligible)
SCALE = 1.0 / 8.0  # 1/sqrt(64)

NM = DM // P       # 8 contraction chunks of 128
NMP = NM // 2      # 4 chunk-pairs (tile granularity)
NHP = HD // P      # 4 head pairs
NSC = S // 512     # 4 s-chunks of 512
NJ = S // P        # 16 key chunks
NJP = NJ // 2      # 8 key chunk-pairs

BF16 = ml_dtypes.bfloat16

_CACHE = {}


def _build_nc():
    import concourse.bass as bass  # noqa: F401
    import concourse.mybir as mybir
    from concourse import bacc, tile
    from contextlib import ExitStack

    BF = mybir.dt.bfloat16
    F32 = mybir.dt.float32
    Exp = mybir.ActivationFunctionType.Exp

    nc = bacc.Bacc("TRN2", target_bir_lowering=False, debug=False, num_swdge_queues=4)

    # pair-layout inputs: row mp*128+p, col sc*1024 + i*512 + t
    qp = nc.dram_tensor("qp", [HD, 4096], BF, kind="ExternalInput")
    kp = nc.dram_tensor("kp", [HD, 4096], BF, kind="ExternalInput")
    vp = nc.dram_tensor("vp", [HD, 4096], BF, kind="ExternalInput")
    # weights, pair layout [mp*128+p, i*512 + h]
    wq = nc.dram_tensor("wq", [HD, 1024], BF, kind="ExternalInput")
    wk = nc.dram_tensor("wk", [HD, 1024], BF, kind="ExternalInput")
    wv = nc.dram_tensor("wv", [HD, 1024], BF, kind="ExternalInput")
    # wo pair layout [hpp*128+p, i*1024 + d]
    wo = nc.dram_tensor("wo", [256, 2048], BF, kind="ExternalInput")
    bq = nc.dram_tensor("bq", [HD], F32, kind="ExternalInput")
    bk = nc.dram_tensor("bk", [HD], F32, kind="ExternalInput")
    out = nc.dram_tensor("out", [S, DM], BF, kind="ExternalOutput")

    with ExitStack() as ctx:
        tc = ctx.enter_context(tile.TileContext(nc))

        const = ctx.enter_context(tc.tile_pool(name="const", bufs=1))
        w_pool = ctx.enter_context(tc.tile_pool(name="w_pool", bufs=4))
        wo_pool = ctx.enter_context(tc.tile_pool(name="wo_pool", bufs=2))
        kin_pool = ctx.enter_context(tc.tile_pool(name="kin_pool", bufs=16))
        qin_pool = ctx.enter_context(tc.tile_pool(name="qin_pool", bufs=8))
        vin_pool = ctx.enter_context(tc.tile_pool(name="vin_pool", bufs=8))
        qh_pool = ctx.enter_context(tc.tile_pool(name="qh_pool", bufs=4))
        vh_pool = ctx.enter_context(tc.tile_pool(name="vh_pool", bufs=8))
        outT_pool = ctx.enter_context(tc.tile_pool(name="outT_pool", bufs=2))
        p_pool = ctx.enter_context(tc.tile_pool(name="p_pool", bufs=4))
        rec_pool = ctx.enter_context(tc.tile_pool(name="rec_pool", bufs=1))
        recb_pool = ctx.enter_context(tc.tile_pool(name="recb_pool", bufs=1))
        tmp_pool = ctx.enter_context(tc.tile_pool(name="tmp_pool", bufs=1))
        stage_pool = ctx.enter_context(tc.tile_pool(name="stage_pool", bufs=2))
        st_ps = ctx.enter_context(tc.tile_pool(name="st_ps", bufs=2, space="PSUM"))
        chain_ps = ctx.enter_context(tc.tile_pool(name="chain_ps", bufs=2, space="PSUM"))
        pv_ps = ctx.enter_context(tc.tile_pool(name="pv_ps", bufs=2, space="PSUM"))

        # constants: one-hot row selector (row 64 -> broadcast rec to 64 rows)
        sel_row = const.tile([P, P], BF, tag="sel", name="sel_row")
        nc.vector.memset(sel_row[:], 0.0)
        nc.vector.memset(sel_row[DK : DK + 1, 0:DK], 1.0)
        recbf_tile = rec_pool.tile([P, 1024], BF, tag="recbf", name="recbf")
        nc.vector.memset(recbf_tile[:], 0.0)
        bq_sb = const.tile([P, NHP], F32, tag="bq", name="bq_sb")
        nc.gpsimd.dma_start(bq_sb[:], bq[:].rearrange("(f p) -> p f", p=P))
        bk_sb = const.tile([P, NHP], F32, tag="bk", name="bk_sb")
        nc.gpsimd.dma_start(bk_sb[:], bk[:].rearrange("(f p) -> p f", p=P))

        # ---- SBUF input tiles [128, 2x512] bf16; k full, q/v 2-slot sc rings
        kin = [
            [
                kin_pool.tile([P, 1024], BF, tag="kin", name=f"k_{mp}_{sc}")
                for sc in range(NSC)
            ]
            for mp in range(NMP)
        ]
        qin = [
            [
                qin_pool.tile([P, 1024], BF, tag="qin", name=f"q_{mp}_{sl}")
                for sl in range(2)
            ]
            for mp in range(NMP)
        ]
        vin = [
            [
                vin_pool.tile([P, 1024], BF, tag="vin", name=f"v_{mp}_{sl}")
                for sl in range(2)
            ]
            for mp in range(NMP)
        ]

        def emit_in_dmas(tiles, handle, sc, eng, ring=False):
            sl = sc % 2 if ring else sc
            for mp in range(NMP):
                eng.dma_start(
                    tiles[mp][sl][:],
                    handle[mp * P : (mp + 1) * P, sc * 1024 : (sc + 1) * 1024],
                )

        def load_w(handle, tag, eng):
            tiles = []
            for mp in range(NMP):
                t = w_pool.tile([P, 1024], BF, tag=tag, name=f"{tag}{mp}")
                eng.dma_start(t[:], handle[mp * P : (mp + 1) * P, :])
                tiles.append(t)
            return tiles

        _q = [nc.gpsimd, nc.scalar, nc.sync]
        _qi = [0]

        def nxq():
            _qi[0] = (_qi[0] + 1) % 3
            return _q[_qi[0]]

        # demand-ordered input DMA (first block needs wk+kp0, wq+qp0, wv+vp0);
        # q/v rings: slots for sc 0/1 upfront, sc 2/3 ride in fillers
        wk_sb = load_w(wk, "wk", nxq())
        emit_in_dmas(kin, kp, 0, nxq())
        wq_sb = load_w(wq, "wq", nxq())
        emit_in_dmas(qin, qp, 0, nxq(), ring=True)
        wv_sb = load_w(wv, "wv", nxq())
        emit_in_dmas(vin, vp, 0, nxq(), ring=True)
        emit_in_dmas(kin, kp, 1, nxq())
        emit_in_dmas(vin, vp, 1, nxq(), ring=True)
        emit_in_dmas(kin, kp, 2, nxq())
        emit_in_dmas(kin, kp, 3, nxq())
        emit_in_dmas(qin, qp, 1, nxq(), ring=True)
        wo_sb = []
        for hpp in range(2):
            t = wo_pool.tile([P, 2048], BF, tag="wo", name=f"wo{hpp}")
            nxq().dma_start(t[:], wo[hpp * P : (hpp + 1) * P, :])
            wo_sb.append(t)

        def pair2(t):
            return t[:].rearrange("p (two c) -> p two c", two=2)

        # ---- vh j-pair tiles [128, 2 x (8 x 66)] bf16, ones at slot col 64
        vh_sb = [
            vh_pool.tile([P, 2 * VW], BF, tag="vh", name=f"vh{i}") for i in range(NJP)
        ]

        def vh4(i):
            return vh_sb[i][:].rearrange("p (two h m) -> p two h m", two=2, h=8)

        for i in range(NJP):
            nc.vector.memset(vh_sb[i][:], 0.0)
            nc.vector.memset(vh4(i)[:, :, :, DK : DK + 1], 1.0)

        def vproj_chain_ops(j):
            sc, half, tq = j // 4, j % 2, (j % 4) * P
            vsl = sc % 2
            cell = {}

            def mk(mp, i):
                def op():
                    if mp == 0 and i == 0:
                        cell["ps"] = chain_ps.tile([P, HD], F32, tag="chps", name="vps")
                    nc.tensor.matmul(
                        cell["ps"][:],
                        lhsT=pair2(vin[mp][vsl])[:, i, tq : tq + P],
                        rhs=pair2(wv_sb[mp])[:, i, :],
                        start=(mp == 0 and i == 0),
                        stop=(mp == NMP - 1 and i == 1),
                    )
                return op

            ops = [mk(mp, i) for mp in range(NMP) for i in range(2)]

            def ev():
                nc.vector.tensor_copy(
                    vh4(j // 2)[:, half, :, 0:DK],
                    cell["ps"][:].rearrange("p (h m) -> p h m", h=8),
                )

            ops.append(ev)
            return ops

        # ---- q/k projections: full bf16 tiles for all head-pairs ----
        qh_sb = [qh_pool.tile([P, S], BF, tag="qh", name=f"qh{h}") for h in range(NHP)]
        kh_sb = [qh_pool.tile([P, S], BF, tag="kh", name=f"kh{h}") for h in range(NHP)]

        def proj_chain_ops(w_sb, x_sb, dst, bias_sb, hp, sc, ring):
            xsl = sc % 2 if ring else sc
            cell = {}

            def mk(mp, i):
                def op():
                    if mp == 0 and i == 0:
                        cell["ps"] = chain_ps.tile([P, HD], F32, tag="chps", name="fps")
                    nc.tensor.matmul(
                        cell["ps"][:],
                        lhsT=pair2(w_sb[mp])[:, i, hp * P : (hp + 1) * P],
                        rhs=pair2(x_sb[mp][xsl])[:, i, :],
                        start=(mp == 0 and i == 0),
                        stop=(mp == NMP - 1 and i == 1),
                    )
                return op

            ops = [mk(mp, i) for mp in range(NMP) for i in range(2)]

            def ev():
                nc.vector.tensor_scalar_add(
                    dst[:, sc * 512 : (sc + 1) * 512],
                    cell["ps"][:],
                    bias_sb[:, hp : hp + 1],
                )

            ops.append(ev)
            return ops

        # ---- outT hp-pair tiles [128, 2 x 2048] bf16 ----
        outT_sb = [
            outT_pool.tile([P, 2 * S], BF, tag="outT", name=f"outT{i}")
            for i in range(2)
        ]

        def fc_chain_ops(ss, ec):
            tsl = slice(ss * P, (ss + 1) * P)
            es = slice(ec * 512, (ec + 1) * 512)
            cell = {}

            def mk(hpp, i):
                def op():
                    if hpp == 0 and i == 0:
                        cell["ps"] = chain_ps.tile([P, HD], F32, tag="chps", name="fcps")
                    nc.tensor.matmul(
                        cell["ps"][:],
                        lhsT=pair2(outT_sb[hpp])[:, i, tsl],
                        rhs=pair2(wo_sb[hpp])[:, i, es],
                        start=(hpp == 0 and i == 0),
                        stop=(hpp == 1 and i == 1),
                    )
                return op

            ops = [mk(hpp, i) for hpp in range(2) for i in range(2)]

            def ev():
                stg = stage_pool.tile([P, 512], BF, tag="stg", name="stg")
                nc.vector.tensor_copy(stg[:], cell["ps"][:])
                nc.gpsimd.dma_start(out[tsl, es], stg[:])

            ops.append(ev)
            return ops

        from collections import deque

        fillers = deque()  # items: (key, op, is_last)
        done = set()

        def push(key, ops):
            for i, op in enumerate(ops):
                fillers.append((key, op, i == len(ops) - 1))

        def drain(n):
            for _ in range(n):
                if not fillers:
                    return
                key, op, last = fillers.popleft()
                op()
                if last:
                    done.add(key)

        def require(key):
            while key not in done and fillers:
                drain(1)

        def kh_chain(hp, sc):
            return proj_chain_ops(wk_sb, kin, kh_sb[hp], bk_sb, hp, sc, False)

        def qh_chain(hp, qc):
            return proj_chain_ops(wq_sb, qin, qh_sb[hp], bq_sb, hp, qc, True)

        # upfront: what block (0,0) needs to start
        for key, ops in (
            (("kh", 0, 0), kh_chain(0, 0)),
            (("qh", 0, 0), qh_chain(0, 0)),
        ):
            for op in ops:
                op()
            done.add(key)

        # fillers: kh for all hps + vproj in demand order, then qh hp1-3 (qc0).
        # v ring: the slot for sc 2/3 is DMA'd after the chains reading it.
        push(("vp", 0), vproj_chain_ops(0))
        push(("vp", 1), vproj_chain_ops(1))
        push(("kh", 0, 1), kh_chain(0, 1))
        push(("vp", 2), vproj_chain_ops(2))
        push(("kh", 0, 2), kh_chain(0, 2))
        push(("vp", 3), vproj_chain_ops(3))
        push(("kh", 0, 3), kh_chain(0, 3))
        push(("vp", 4), vproj_chain_ops(4))
        push(("kh", 1, 0), kh_chain(1, 0))
        push(("vp", 5), vproj_chain_ops(5))
        push(("kh", 1, 1), kh_chain(1, 1))
        push(("vp", 6), vproj_chain_ops(6))
        push(("vp", 7), vproj_chain_ops(7))
        push(("vdma", 2), [lambda: emit_in_dmas(vin, vp, 2, nc.gpsimd, ring=True)])
        push(("kh", 1, 2), kh_chain(1, 2))
        push(("vp", 8), vproj_chain_ops(8))
        push(("kh", 1, 3), kh_chain(1, 3))
        push(("vp", 9), vproj_chain_ops(9))
        push(("kh", 2, 0), kh_chain(2, 0))
        push(("vp", 10), vproj_chain_ops(10))
        push(("vp", 11), vproj_chain_ops(11))
        push(("vdma", 3), [lambda: emit_in_dmas(vin, vp, 3, nc.gpsimd, ring=True)])
        push(("kh", 2, 1), kh_chain(2, 1))
        push(("vp", 12), vproj_chain_ops(12))
        push(("kh", 2, 2), kh_chain(2, 2))
        push(("vp", 13), vproj_chain_ops(13))
        push(("kh", 2, 3), kh_chain(2, 3))
        push(("vp", 14), vproj_chain_ops(14))
        push(("kh", 3, 0), kh_chain(3, 0))
        push(("vp", 15), vproj_chain_ops(15))
        push(("kh", 3, 1), kh_chain(3, 1))
        push(("kh", 3, 2), kh_chain(3, 2))
        push(("kh", 3, 3), kh_chain(3, 3))
        push(("qh", 1, 0), qh_chain(1, 0))
        push(("qh", 2, 0), qh_chain(2, 0))
        push(("qh", 3, 0), qh_chain(3, 0))
        # q ring: the sc=2 slot frees once the qh(*, 0) chains are done
        push(("qdma", 2), [lambda: emit_in_dmas(qin, qp, 2, nc.gpsimd, ring=True)])

        # ---- attention block: ST bf16 pairs + EXP + PV, fillers between ----
        carry = []

        def attn_block(hp, qc, budget, lagp=2):
            for sc in range(NSC):
                require(("kh", hp, sc))
            require(("qh", hp, qc))
            qs = slice(qc * 512, (qc + 1) * 512)
            state = {}
            p_tiles = {}

            def emit_st(j):
                ks = slice(j * P, (j + 1) * P)
                st = st_ps.tile([P, 1024], F32, tag="stps", name="stps")
                nc.tensor.matmul(
                    st[:, 0:512],
                    lhsT=kh_sb[hp][0:64, ks],
                    rhs=qh_sb[hp][0:64, qs],
                    start=True,
                    stop=True,
                    tile_position=(0, 0),
                )
                nc.tensor.matmul(
                    st[:, 512:1024],
                    lhsT=kh_sb[hp][64:128, ks],
                    rhs=qh_sb[hp][64:128, qs],
                    start=True,
                    stop=True,
                    tile_position=(64, 0),
                )
                jp, jh = j // 2, j % 2
                if jh == 0:
                    p_tiles[jp] = p_pool.tile([P, 2048], BF, tag="p", name="ppair")
                pt4 = p_tiles[jp][:].rearrange("p (h j n) -> p h j n", h=2, j=2)
                nc.scalar.activation(
                    pt4[:, :, jh, :],
                    st[:].rearrange("p (h n) -> p h n", h=2),
                    Exp,
                    scale=SCALE,
                )

            def emit_pv(jp):
                require(("vp", 2 * jp))
                require(("vp", 2 * jp + 1))
                if "P0" not in state:
                    state["P0"] = pv_ps.tile([P, 512], F32, tag="pvps", name="P0ps")
                    state["P1"] = pv_ps.tile([P, 512], F32, tag="pvps", name="P1ps")
                pt = p_tiles.pop(jp)
                pt4 = pt[:].rearrange("p (h j n) -> p h j n", h=2, j=2)
                for i in range(2):
                    first, last = (jp == 0 and i == 0), (jp == NJP - 1 and i == 1)
                    for h in range(2):
                        nc.tensor.matmul(
                            state["P0" if h == 0 else "P1"][:, :],
                            lhsT=vh4(jp)[:, i, 2 * hp + h, :],
                            rhs=pt4[:, h, i, :],
                            start=first,
                            stop=last,
                            skip_group_check=True,
                        )

            for i in range(NJP):
                # both j's ST pairs back-to-back: one contiguous K=64 tile
                # stretch per pair-step halves the PE 64<->128 mode switches
                emit_st(2 * i)
                emit_st(2 * i + 1)
                for _ in range(4):
                    if carry:
                        op = carry.pop(0)
                        if op is not None:
                            op()
                if i >= lagp:
                    emit_pv(i - lagp)
                if not carry:
                    drain(budget[i] if isinstance(budget, list) else budget)

            def mk_pv(jp):
                return lambda: emit_pv(jp)

            def mk_norm():
                cellN = {}
                half = hp % 2
                ot = outT_sb[hp // 2]

                def ot2(lo, hi):
                    return (
                        ot[:]
                        .rearrange("p (two s) -> p two s", two=2)[lo:hi, half, qs]
                    )

                def evac_p0():
                    E0 = tmp_pool.tile([P, 512], F32, tag="E0", name="E0")
                    cellN["E0"] = E0
                    nc.vector.tensor_copy(E0[0 : DK + 1, :], state["P0"][0 : DK + 1, :])

                def evac_p1():
                    E1 = tmp_pool.tile([P, 512], F32, tag="E1", name="E1")
                    cellN["E1"] = E1
                    nc.vector.tensor_copy(E1[0 : DK + 1, :], state["P1"][0 : DK + 1, :])

                def recips():
                    # full [0:65] so the custom-DVE op starts at partition 0;
                    # rows 0:64 compute junk reciprocals that are never read
                    rec = rec_pool.tile([P, 1024], F32, tag="rec", name="rec")
                    cellN["rec"] = rec
                    nc.vector.reciprocal_approx_fast(
                        rec[0 : DK + 1, 0:512], cellN["E0"][0 : DK + 1, :]
                    )
                    nc.vector.reciprocal_approx_fast(
                        rec[0 : DK + 1, 512:1024], cellN["E1"][0 : DK + 1, :]
                    )

                def to_bf():
                    nc.vector.tensor_copy(
                        recbf_tile[DK : DK + 1, :], cellN["rec"][DK : DK + 1, :]
                    )

                def bcast_mm():
                    r0 = chain_ps.tile([P, HD], F32, tag="chps", name="rb0")
                    r1 = chain_ps.tile([P, HD], F32, tag="chps", name="rb1")
                    cellN["rb0"], cellN["rb1"] = r0, r1
                    nc.tensor.matmul(
                        r0[:], lhsT=sel_row[:], rhs=recbf_tile[:, 0:512],
                        start=True, stop=True,
                    )
                    nc.tensor.matmul(
                        r1[:], lhsT=sel_row[:], rhs=recbf_tile[:, 512:1024],
                        start=True, stop=True,
                    )

                def evac_recb():
                    recb_sb = recb_pool.tile([P, 1024], BF, tag="recb", name="recb")
                    cellN["recb"] = recb_sb
                    nc.vector.tensor_copy(recb_sb[0:DK, 0:512], cellN["rb0"][0:DK, :])
                    nc.vector.tensor_copy(
                        recb_sb[0:DK, 512:1024], cellN["rb1"][0:DK, :]
                    )

                def mul_e():
                    nc.vector.tensor_mul(
                        ot2(0, DK),
                        cellN["E0"][0:DK, :],
                        cellN["recb"][0:DK, 0:512],
                    )

                def mul_o():
                    tmp = tmp_pool.tile([P, 512], BF, tag="tmp", name="tmp")
                    cellN["tmp"] = tmp
                    nc.vector.tensor_mul(
                        tmp[0:DK, :],
                        cellN["E1"][0:DK, :],
                        cellN["recb"][0:DK, 512:1024],
                    )

                def shift():
                    nc.sync.dma_start(ot2(DK, P), cellN["tmp"][0:DK, :])

                return [
                    evac_p0,
                    evac_p1,
                    recips,
                    None,
                    to_bf,
                    None,
                    None,
                    bcast_mm,
                    evac_recb,
                    mul_e,
                    mul_o,
                    shift,
                ]

            return [mk_pv(jp) for jp in range(NJP - lagp, NJP)] + mk_norm()

        for qc in range(NSC):
            for hp in range(NHP):
                if qc == 0 and hp == 0:
                    # DMA-limited ramp: hold fillers while inputs land
                    carry = attn_block(hp, qc, [0, 0, 2, 4, 7, 7, 8, 8], lagp=3)
                else:
                    carry = attn_block(hp, qc, 8 if qc == 0 else 3)
                if hp == 0 and qc > 0:
                    # previous qc's outT is complete; queue its fc chains
                    for ss in range((qc - 1) * 4, (qc - 1) * 4 + 4):
                        push(("fc", ss, 0), fc_chain_ops(ss, 0))
                        push(("fc", ss, 1), fc_chain_ops(ss, 1))
                if qc < NSC - 1:
                    push(("qh", hp, qc + 1), qh_chain(hp, qc + 1))
                    if hp == NHP - 1 and qc + 3 <= NSC - 1:
                        # q ring: the slot for sc=qc+3 frees after qh(*, qc+1)
                        push(
                            ("qdma", qc + 3),
                            [
                                lambda sc=qc + 3: emit_in_dmas(
                                    qin, qp, sc, nc.gpsimd, ring=True
                                )
                            ],
                        )
        for op in carry:
            if op is not None:
                op()
        for ss in range(12, 16):
            push(("fc", ss, 0), fc_chain_ops(ss, 0))
            push(("fc", ss, 1), fc_chain_ops(ss, 1))
        while fillers:
            drain(1)

    nc.compile()
    return nc


def _get_nc():
    if "nc" not in _CACHE:
        _CACHE["nc"] = _build_nc()
    return _CACHE["nc"]


def _pair_inputs(xT):
    # [1024, 2048] -> [512, 4096]: row mp*128+p, col sc*1024 + i*512 + t
    return (
        xT.reshape(NMP, 2, P, NSC, 512)
        .transpose(0, 2, 3, 1, 4)
        .reshape(HD, 4096)
        .astype(BF16)
    )


def kernel(q, k, v, Wq, bq, Wk, bk, Wv, bv, Wo, bo):
    from concourse.bass_utils import run_bass_kernel_spmd

    q, k, v = (np.asarray(x, np.float32) for x in (q, k, v))
    Wq, bq, Wk, bk, Wv, bv, Wo, bo = (
        np.asarray(x, np.float32) for x in (Wq, bq, Wk, bk, Wv, bv, Wo, bo)
    )

    def wpair(W, t):
        Ws = W[:, t * HD : (t + 1) * HD]
        return (
            Ws.reshape(NMP, 2, P, HD).transpose(0, 2, 1, 3).reshape(HD, 1024).astype(BF16)
        )

    def wopair(t):
        Ws = Wo[t * HD : (t + 1) * HD, :]
        return (
            Ws.reshape(2, 2, P, DM).transpose(0, 2, 1, 3).reshape(256, 2048).astype(BF16)
        )

    in_maps = []
    for c in range(NCORES):
        b, t = c // 2, c % 2
        hs = slice(t * HD, (t + 1) * HD)
        in_maps.append(
            {
                "qp": _pair_inputs(q[b].T),
                "kp": _pair_inputs(k[b].T),
                "vp": _pair_inputs(v[b].T),
                "wq": wpair(Wq, t),
                "wk": wpair(Wk, t),
                "wv": wpair(Wv, t),
                "wo": wopair(t),
                "bq": np.ascontiguousarray(bq[hs]),
                "bk": np.ascontiguousarray(bk[hs]),
            }
        )

    nc = _get_nc()
    trace = os.environ.get("KERNEL_TRACE", "0") == "1"
    res = run_bass_kernel_spmd(
        nc, in_maps, core_ids=list(range(NCORES)), trace=trace
    )
    if trace:
        print(f"HW exec time: {res.exec_time_ns} ns")

    host_bias = (bv @ Wo + bo).astype(np.float32)
    full = np.empty((NB, S, DM), np.float32)
    for b in range(NB):
        full[b] = (
            res.results[2 * b]["out"].astype(np.float32)
            + res.results[2 * b + 1]["out"].astype(np.float32)
            + host_bias
        )
    return full
